# revision 4
# baseline (speedup 1.0000x reference)
"""Trainium2 Bass kernel for nn_MessageEncoderLongformer.

Mathematical reduction: the module's output is
    y = tanh(attn[:,0] @ wp + bp) @ wfc + bfc
and attn[:,0] depends only on the CLS token's global-attention path
(position 0 is a global token, so row 0 of the attention output comes from
the dedicated global projections):
    qg0   = (x[:,0] @ wqg + bqg) * scale                       [B, DM]
    s[s,h]= qg0[b,h,:] . (x[b,s] @ wkg)[h,:]   (+ bkg term, constant in s,
                                                cancels in softmax)
    p     = softmax_s(s)
    og[h] = (sum_s p[s,h] x[b,s]) @ wvg[:,hb] + bvg[hb]        [B, DM]
    attn0 = og @ wo + bo; pooled = tanh(attn0 @ wp + bp); y = pooled @ wfc + bfc
The sliding-window attention is dead code for this output.

Factorization: s = x @ U with U = wkg @ Q, where Q[j,h] = qg0[j] iff
j//64 == h (block diagonal). og's inner sum is C[h,:] = sum_s p[s,h] x[b,s].
Only two S-sized matmuls remain: scores ([768,12] applied to x) and
C = p^T @ [x | 1] (the ones column gives the softmax denominators).

Distribution: 8 cores = (2 batches) x (4 sequence quarters). Each core
computes its quarter's scores/exp/C partials; one AllReduce per batch group
of 4 cores combines C and the denominators; the tiny tail (og -> attn0 ->
pooled -> y) is computed redundantly per group, and the host takes core 0's
and core 4's outputs.
"""

import os
import numpy as np
import ml_dtypes

import concourse.bacc as bacc
import concourse.mybir as mybir
import concourse.tile as tile
from concourse.bass_utils import run_bass_kernel_spmd
from concourse.masks import make_identity

F32 = mybir.dt.float32
F32R = mybir.dt.float32r
BF16 = mybir.dt.bfloat16
AF = mybir.ActivationFunctionType
MUL = mybir.AluOpType.mult

B, S, DM, H, HD = 2, 4096, 768, 12, 64
OUT = 512
SCALE = 1.0 / 8.0
P = 128
KC = DM // P            # 6 chunks of the 768 contraction dim
NCORES = 8
SHARD = S // 4          # 1024 sequence rows per core
SC = SHARD // P         # 8 s-chunks per core
NW = 772                # 768 cols + ones col (at 768) + pad
KE = 7                  # extended contraction chunks (768 + bias row -> 896)

_DEBUG_OUTS = os.environ.get("BASS_KERNEL_DEBUG", "0") == "1"


def build_program(debug=False, debug_outs=_DEBUG_OUTS):
    nc = bacc.Bacc("TRN2", target_bir_lowering=False, debug=debug,
                   num_devices=NCORES)

    # ---- parameters (per core) ----
    xs_p = nc.declare_dram_parameter("xs", [SC, P, NW], F32, isOutput=False)
    x0t_p = nc.declare_dram_parameter("x0t", [KE, P, 1], F32, isOutput=False)
    wqg_p = nc.declare_dram_parameter("wqg_s", [KE, P, DM], BF16, isOutput=False)
    wkgt_p = nc.declare_dram_parameter("wkgt", [KC, P, DM], BF16, isOutput=False)
    wvg_p = nc.declare_dram_parameter("wvg_e", [KE, P, DM], BF16, isOutput=False)
    wo_p = nc.declare_dram_parameter("wo_e", [KE, P, DM], BF16, isOutput=False)
    wp_p = nc.declare_dram_parameter("wp_e", [KE, P, DM], BF16, isOutput=False)
    wfc_p = nc.declare_dram_parameter("wfc_e", [KE, P, OUT], BF16, isOutput=False)
    hmask_p = nc.declare_dram_parameter("hmask", [12, DM], F32, isOutput=False)
    y_p = nc.declare_dram_parameter("y", [1, OUT], F32, isOutput=True)
    dbg = {}
    if debug_outs:
        for name, shape, dt in [
            ("dbg_qg0", [1, DM], BF16), ("dbg_ut", [12, DM], F32),
            ("dbg_pt", [12, SHARD], F32), ("dbg_cext", [12, NW], F32),
            ("dbg_arout", [12, NW], F32), ("dbg_og", [1, DM], F32),
            ("dbg_a0", [1, DM], F32), ("dbg_pl", [1, DM], F32),
        ]:
            dbg[name] = nc.declare_dram_parameter(name, shape, dt, isOutput=True)

    groups = [[0, 1, 2, 3], [4, 5, 6, 7]]

    with tile.TileContext(nc) as tc:
        with tc.tile_pool(name="sb", bufs=1) as sb, \
             tc.tile_pool(name="pp", bufs=1, space="PSUM") as pp, \
             tc.tile_pool(name="dram", bufs=1, space="DRAM") as dram:

            def acc_tile(shape, name):
                return pp.tile(shape, F32, name=name, tag="acc", bufs=3,
                               padded_shape=[shape[0], 512])

            def tp_tile(shape, dt, name):
                return pp.tile(shape, dt, name=name, tag="tp", bufs=3,
                               padded_shape=[shape[0], 128])

            # ---------- DMAs (program order sets issue/queue order) ----------
            x0t_sb = sb.tile([P, KE, 1], F32, name="x0t_sb")
            nc.sync.dma_start(x0t_sb[:], x0t_p.rearrange("k p one -> p k one"))
            wqg_sb = sb.tile([P, KE, DM], BF16, name="wqg_sb")
            wkgt_sb = sb.tile([P, KC, DM], BF16, name="wkgt_sb")
            for j in range(KE):
                nc.sync.dma_start(wqg_sb[:, j, :], wqg_p[j])
            for j in range(KC):
                nc.sync.dma_start(wkgt_sb[:, j, :], wkgt_p[j])
            xs_sb = sb.tile([P, SC, NW], F32, name="xs_sb")
            for s in range(SC):
                nc.sync.dma_start(xs_sb[:, s, :], xs_p[s])
            wvg_sb = sb.tile([P, KE, DM], BF16, name="wvg_sb")
            wo_sb = sb.tile([P, KE, DM], BF16, name="wo_sb")
            wp_sb = sb.tile([P, KE, DM], BF16, name="wp_sb")
            wfc_sb = sb.tile([P, KE, OUT], BF16, name="wfc_sb")
            for j in range(KE):
                nc.sync.dma_start(wvg_sb[:, j, :], wvg_p[j])
            for j in range(KE):
                nc.sync.dma_start(wo_sb[:, j, :], wo_p[j])
            for j in range(KE):
                nc.sync.dma_start(wp_sb[:, j, :], wp_p[j])
            for j in range(KE):
                nc.sync.dma_start(wfc_sb[:, j, :], wfc_p[j])
            hmask_sb = sb.tile([12, DM], F32, name="hmask_sb")
            nc.sync.dma_start(hmask_sb[:], hmask_p[:])

            # ---------- constants ----------
            ident = sb.tile([P, P], F32, name="ident")
            make_identity(nc, ident[:])
            identr = sb.tile([P, P], F32R, name="identr")
            nc.vector.tensor_copy(identr[:], ident[:])
            ident_bf = sb.tile([12, 12], BF16, name="ident_bf")
            nc.vector.tensor_copy(ident_bf[:], ident[0:12, 0:12])
            one11_f = sb.tile([1, 1], F32, name="one11_f")
            nc.vector.memset(one11_f[:], 1.0)
            one11_bf = sb.tile([1, 1], BF16, name="one11_bf")
            nc.vector.memset(one11_bf[:], 1.0)
            ones_r12 = sb.tile([1, 12], BF16, name="ones_r12")
            nc.vector.memset(ones_r12[:], 1.0)
            ones12_f = sb.tile([12, 1], F32, name="ones12_f")
            nc.vector.memset(ones12_f[:], 1.0)
            e0_bf = sb.tile([P, 1], BF16, name="e0_bf")
            nc.vector.memset(e0_bf[:], 0.0)
            nc.vector.memset(e0_bf[0:1, :], 1.0)

            # ---------- stage A: qg0 = x0_ext @ (wqg*scale | bqg*scale) ----------
            x0t_bf = sb.tile([P, KE, 1], BF16, name="x0t_bf")
            nc.vector.tensor_copy(x0t_bf[:], x0t_sb[:])
            qg0_ps0 = acc_tile([1, 512], "qg0_ps0")
            qg0_ps1 = acc_tile([1, 256], "qg0_ps1")
            for j in range(KE):
                st, sp = (j == 0), (j == KE - 1)
                nc.tensor.matmul(qg0_ps0[:], x0t_bf[:, j, :], wqg_sb[:, j, 0:512],
                                 start=st, stop=sp)
                nc.tensor.matmul(qg0_ps1[:], x0t_bf[:, j, :], wqg_sb[:, j, 512:768],
                                 start=st, stop=sp)
            qg0_sb = sb.tile([1, DM], BF16, name="qg0_sb")
            nc.vector.tensor_copy(qg0_sb[0:1, 0:512], qg0_ps0[:])
            nc.vector.tensor_copy(qg0_sb[0:1, 512:768], qg0_ps1[:])
            if debug_outs:
                nc.sync.dma_start(dbg["dbg_qg0"][:], qg0_sb[:])

            # ---------- stage B: Q (block-diag) and UT = Q^T @ wkg^T ----------
            q_sb = sb.tile([P, KC, 12], BF16, name="q_sb")
            nc.vector.memset(q_sb[:], 0.0)
            for j in range(KC):
                qt_ps = tp_tile([P, 1], F32, f"qt_ps{j}")
                nc.tensor.matmul(qt_ps[:], qg0_sb[0:1, j * P:(j + 1) * P],
                                 one11_bf[:], start=True, stop=True)
                for m in range(2):
                    h = 2 * j + m
                    nc.vector.tensor_copy(q_sb[64 * m:64 * (m + 1), j, h:h + 1],
                                          qt_ps[64 * m:64 * (m + 1), :])
            ut_ps0 = acc_tile([12, 512], "ut_ps0")
            ut_ps1 = acc_tile([12, 256], "ut_ps1")
            for j in range(KC):
                st, sp = (j == 0), (j == KC - 1)
                nc.tensor.matmul(ut_ps0[:], q_sb[:, j, :], wkgt_sb[:, j, 0:512],
                                 start=st, stop=sp)
                nc.tensor.matmul(ut_ps1[:], q_sb[:, j, :], wkgt_sb[:, j, 512:768],
                                 start=st, stop=sp)
            ut_sb = sb.tile([12, DM], F32R, name="ut_sb")
            nc.vector.tensor_copy(ut_sb[:, 0:512], ut_ps0[:])
            nc.vector.tensor_copy(ut_sb[:, 512:768], ut_ps1[:])
            if debug_outs:
                nc.sync.dma_start(dbg["dbg_ut"][:], ut_sb[:])
            # U chunks [128, 12] (k on partitions) via PE transpose
            u_sb = sb.tile([P, KC, 12], F32R, name="u_sb")
            for j in range(KC):
                utt_ps = tp_tile([P, 12], F32R, f"utt_ps{j}")
                nc.tensor.transpose(utt_ps[:], ut_sb[:, j * P:(j + 1) * P],
                                    identr[0:12, 0:12])
                nc.vector.tensor_copy(u_sb[:, j, :], utt_ps[:])

            # ---------- stage C: per s-chunk cast + transpose ----------
            xr_sb = sb.tile([P, SC, NW], F32R, name="xr_sb")
            xt_sb = sb.tile([P, KC, SHARD], F32R, name="xt_sb")
            for s in range(SC):
                nc.scalar.activation(xr_sb[:, s, :], xs_sb[:, s, :], AF.Copy)
                for j in range(KC):
                    tp_ps = tp_tile([P, P], F32R, f"tp_ps{s}_{j}")
                    nc.tensor.transpose(tp_ps[:], xr_sb[:, s, j * P:(j + 1) * P],
                                        identr[:])
                    nc.vector.tensor_copy(xt_sb[:, j, s * P:(s + 1) * P], tp_ps[:])

            # ---------- stage D: scoresT + exp + p-transpose ----------
            pt_sb = sb.tile([12, SHARD], F32, name="pt_sb")
            for sh in range(2):
                sc_ps = acc_tile([12, 512], f"sc_ps{sh}")
                for j in range(KC):
                    nc.tensor.matmul(sc_ps[:], u_sb[:, j, :],
                                     xt_sb[:, j, sh * 512:(sh + 1) * 512],
                                     start=(j == 0), stop=(j == KC - 1))
                nc.scalar.activation(pt_sb[:, sh * 512:(sh + 1) * 512], sc_ps[:],
                                     AF.Exp)
            if debug_outs:
                nc.sync.dma_start(dbg["dbg_pt"][:], pt_sb[:])
            p_sb = sb.tile([P, SC, 12], F32R, name="p_sb")
            for s in range(SC):
                ptt_ps = tp_tile([P, 12], F32, f"ptt_ps{s}")
                nc.tensor.transpose(ptt_ps[:], pt_sb[:, s * P:(s + 1) * P],
                                    ident[0:12, 0:12])
                nc.vector.tensor_copy(p_sb[:, s, :], ptt_ps[:])

            # ---------- stage E: Cext = p^T @ [x | 1] ----------
            cx_ps0 = acc_tile([12, 512], "cx_ps0")
            cx_ps1 = acc_tile([12, 260], "cx_ps1")
            for s in range(SC):
                st, sp = (s == 0), (s == SC - 1)
                nc.tensor.matmul(cx_ps0[:], p_sb[:, s, :], xr_sb[:, s, 0:512],
                                 start=st, stop=sp)
                nc.tensor.matmul(cx_ps1[:], p_sb[:, s, :], xr_sb[:, s, 512:772],
                                 start=st, stop=sp)
            car_sb = sb.tile([12, NW], F32, name="car_sb")
            nc.vector.tensor_copy(car_sb[:, 0:512], cx_ps0[:])
            nc.vector.tensor_copy(car_sb[:, 512:772], cx_ps1[:])
            if debug_outs:
                nc.sync.dma_start(dbg["dbg_cext"][:], car_sb[:])

            # ---------- AllReduce over the 4 cores of this batch ----------
            arin_d = dram.tile([12, NW], F32, name="arin_d")
            arout_d = dram.tile([12, NW], F32, name="arout_d")
            nc.sync.dma_start(arin_d[:], car_sb[:])
            nc.gpsimd.collective_compute(
                "AllReduce", mybir.AluOpType.add,
                replica_groups=groups, ins=[arin_d.opt()], outs=[arout_d.opt()])
            if debug_outs:
                nc.gpsimd.dma_start(dbg["dbg_arout"][:], arout_d[:])

            # ---------- tail: normalize C, og ----------
            ar_sb = sb.tile([12, NW], F32, name="ar_sb")
            nc.sync.dma_start(ar_sb[:], arout_d[:])
            zrec_sb = sb.tile([12, 1], F32, name="zrec_sb")
            nc.vector.reciprocal(zrec_sb[:], ar_sb[:, 768:769])
            arn_sb = sb.tile([12, DM], BF16, name="arn_sb")
            nc.vector.tensor_scalar(arn_sb[:], ar_sb[:, 0:768], zrec_sb[:], None,
                                    MUL)
            # ĈT chunks [128, 12] bf16 via PE transpose
            ctn_sb = sb.tile([P, KC, 12], BF16, name="ctn_sb")
            for j in range(KC):
                ct_ps = tp_tile([P, 12], BF16, f"ct_ps{j}")
                nc.tensor.transpose(ct_ps[:], arn_sb[:, j * P:(j + 1) * P],
                                    ident_bf[:])
                nc.vector.tensor_copy(ctn_sb[:, j, :], ct_ps[:])
            ogp_ps0 = acc_tile([12, 512], "ogp_ps0")
            ogp_ps1 = acc_tile([12, 256], "ogp_ps1")
            for j in range(KE):
                st, sp = (j == 0), (j == KE - 1)
                lhs = ctn_sb[:, j, :] if j < KC else ones_r12[:]
                rhs0 = wvg_sb[:, j, 0:512] if j < KC else wvg_sb[0:1, j, 0:512]
                rhs1 = wvg_sb[:, j, 512:768] if j < KC else wvg_sb[0:1, j, 512:768]
                nc.tensor.matmul(ogp_ps0[:], lhs, rhs0, start=st, stop=sp)
                nc.tensor.matmul(ogp_ps1[:], lhs, rhs1, start=st, stop=sp)
            masked_sb = sb.tile([12, DM], F32, name="masked_sb")
            nc.vector.tensor_tensor(masked_sb[:, 0:512], ogp_ps0[:],
                                    hmask_sb[:, 0:512], MUL)
            nc.vector.tensor_tensor(masked_sb[:, 512:768], ogp_ps1[:],
                                    hmask_sb[:, 512:768], MUL)
            og_ps0 = acc_tile([1, 512], "og_ps0")
            og_ps1 = acc_tile([1, 256], "og_ps1")
            nc.tensor.matmul(og_ps0[:], ones12_f[:], masked_sb[:, 0:512],
                             start=True, stop=True)
            nc.tensor.matmul(og_ps1[:], ones12_f[:], masked_sb[:, 512:768],
                             start=True, stop=True)
            og_sb = sb.tile([1, DM], F32, name="og_sb")
            nc.vector.tensor_copy(og_sb[0:1, 0:512], og_ps0[:])
            nc.vector.tensor_copy(og_sb[0:1, 512:768], og_ps1[:])
            if debug_outs:
                nc.sync.dma_start(dbg["dbg_og"][:], og_sb[:])

            # ---------- tail: attn0, pooled, y ----------
            def row_to_col_bf16(row_sb, name):
                """[1, 768] f32 -> [128, 6, 1] bf16 via K=1 outer-product."""
                col = sb.tile([P, KC, 1], BF16, name=name)
                for j in range(KC):
                    v_ps = tp_tile([P, 1], F32, f"{name}_ps{j}")
                    nc.tensor.matmul(v_ps[:], row_sb[0:1, j * P:(j + 1) * P],
                                     one11_f[:], start=True, stop=True)
                    nc.vector.tensor_copy(col[:, j, :], v_ps[:])
                return col

            def mat_vec(w_sb, colv, n, name):
                """psum pair [1, n] = [colv; e0]^T @ w_ext (bias row folded)."""
                n0 = min(n, 512)
                ps0 = acc_tile([1, n0], f"{name}0")
                ps1 = acc_tile([1, n - n0], f"{name}1") if n > n0 else None
                for j in range(KE):
                    st, sp = (j == 0), (j == KE - 1)
                    lhs = colv[:, j, :] if j < KC else e0_bf[:]
                    nc.tensor.matmul(ps0[:], lhs, w_sb[:, j, 0:n0],
                                     start=st, stop=sp)
                    if ps1 is not None:
                        nc.tensor.matmul(ps1[:], lhs, w_sb[:, j, n0:n],
                                         start=st, stop=sp)
                return ps0, ps1

            ogt = row_to_col_bf16(og_sb, "ogt")
            a0_ps0, a0_ps1 = mat_vec(wo_sb, ogt, DM, "a0_ps")
            a0_sb = sb.tile([1, DM], F32, name="a0_sb")
            nc.vector.tensor_copy(a0_sb[0:1, 0:512], a0_ps0[:])
            nc.vector.tensor_copy(a0_sb[0:1, 512:768], a0_ps1[:])
            if debug_outs:
                nc.sync.dma_start(dbg["dbg_a0"][:], a0_sb[:])
            a0t = row_to_col_bf16(a0_sb, "a0t")
            pl_ps0, pl_ps1 = mat_vec(wp_sb, a0t, DM, "pl_ps")
            pl_sb = sb.tile([1, DM], F32, name="pl_sb")
            nc.scalar.activation(pl_sb[0:1, 0:512], pl_ps0[:], AF.Tanh)
            nc.scalar.activation(pl_sb[0:1, 512:768], pl_ps1[:], AF.Tanh)
            if debug_outs:
                nc.sync.dma_start(dbg["dbg_pl"][:], pl_sb[:])
            plt = row_to_col_bf16(pl_sb, "plt")
            y_ps0, _ = mat_vec(wfc_sb, plt, OUT, "y_ps")
            y_sb = sb.tile([1, OUT], F32, name="y_sb")
            nc.vector.tensor_copy(y_sb[:], y_ps0[:])
            nc.sync.dma_start(y_p[:], y_sb[:])

    nc.compile()
    return nc


def _prep_inputs(x, wq, bq, wk, bk, wv, bv, wqg, bqg, wkg, bkg, wvg, bvg,
                 wo, bo, wp, bp, wfc, bfc):
    """Host-side marshalling: shard, pad, pre-transpose, fold biases/scale."""
    bf = ml_dtypes.bfloat16

    def ext(w, b, ncols):
        we = np.zeros((KE * P, ncols), np.float32)
        we[:DM] = w
        we[DM] = b
        return np.ascontiguousarray(we.reshape(KE, P, ncols)).astype(bf)

    wqg_s = ext(np.asarray(wqg) * SCALE, np.asarray(bqg) * SCALE, DM)
    wkgt = np.ascontiguousarray(np.asarray(wkg).T.reshape(KC, P, DM)).astype(bf)
    wvg_e = ext(np.asarray(wvg), np.asarray(bvg), DM)
    wo_e = ext(np.asarray(wo), np.asarray(bo), DM)
    wp_e = ext(np.asarray(wp), np.asarray(bp), DM)
    wfc_e = ext(np.asarray(wfc), np.asarray(bfc), OUT)
    hmask = np.zeros((12, DM), np.float32)
    for h in range(H):
        hmask[h, 64 * h:64 * (h + 1)] = 1.0
    x = np.asarray(x)

    in_maps = []
    for core in range(NCORES):
        b, q = core // 4, core % 4
        xsh = np.zeros((SHARD, NW), np.float32)
        xsh[:, :DM] = x[b, q * SHARD:(q + 1) * SHARD, :]
        xsh[:, DM] = 1.0
        x0t = np.zeros((KE * P, 1), np.float32)
        x0t[:DM, 0] = x[b, 0, :]
        x0t[DM, 0] = 1.0
        in_maps.append({
            "xs": np.ascontiguousarray(xsh.reshape(SC, P, NW)),
            "x0t": np.ascontiguousarray(x0t.reshape(KE, P, 1)),
            "wqg_s": wqg_s, "wkgt": wkgt, "wvg_e": wvg_e,
            "wo_e": wo_e, "wp_e": wp_e, "wfc_e": wfc_e, "hmask": hmask,
        })
    return in_maps


_NC_CACHE = {}


def _get_program(debug=False):
    key = (debug, _DEBUG_OUTS)
    if key not in _NC_CACHE:
        _NC_CACHE[key] = build_program(debug=debug)
    return _NC_CACHE[key]


def kernel(**inputs) -> np.ndarray:
    in_maps = _prep_inputs(**inputs)
    nc = _get_program(debug=False)
    trace = os.environ.get("BASS_KERNEL_TRACE", "0") == "1"
    kw = {}
    if trace:
        kw = dict(trace=True, trace_cores=list(range(NCORES)))
    r = run_bass_kernel_spmd(nc, in_maps, list(range(NCORES)), **kw)
    kernel.last_results = r
    y = np.stack([r.results[0]["y"][0], r.results[4]["y"][0]]).astype(np.float32)
    return y


def _dbg_names():
    return ["dbg_qg0", "dbg_ut", "dbg_pt", "dbg_cext", "dbg_arout",
            "dbg_og", "dbg_a0", "dbg_pl"]


def run_sim(**inputs):
    """CoreSim path for fast correctness iteration (no hardware)."""
    from concourse.bass_interp import MultiCoreSim
    in_maps = _prep_inputs(**inputs)
    nc = _get_program(debug=True)
    sim = MultiCoreSim(nc, NCORES, num_workers=min(8, os.cpu_count() or 1))
    for i in range(NCORES):
        for name, arr in in_maps[i].items():
            sim.cores[i].tensor(name)[:] = arr
    sim.simulate()
    results = []
    outs = ["y"] + (list(_dbg_names()) if _DEBUG_OUTS else [])
    for i in range(NCORES):
        results.append({name: np.array(sim.cores[i].tensor(name))
                        for name in outs})
    run_sim.last_results = results
    return np.stack([results[0]["y"][0], results[4]["y"][0]]).astype(np.float32)
